# revision 7
# baseline (speedup 1.0000x reference)
"""Int4 dequant matmul kernel for Trainium2 (8 NeuronCores, tensor-parallel).

Computes y = x @ W.T where W = (nibbles(weight_packed) - zero) * scale,
x: (4096, 4096) f32, weight_packed: (11008, 2048) u8, y: (4096, 11008) f32.

Strategy (column-parallel over out_features, x replicated):
  y[t,o] = scale[o] * (sum_k x[t,k]*(n[o,k]-7.5)) + scale[o]*(7.5-zero[o])*S[t]
with S[t] = sum_k x[t,k].  The matmul runs in bf16 with EXACT weights
(n-7.5 is exactly representable in bf16 after a power-of-2 chunk scaling),
so the only error source is bf16 rounding of x (zero-mean weights make the
systematic term vanish).

Contraction permutation: weight bytes are viewed as u32 words; SBUF weight
staging is loaded with a 16-byte-granular transposed DMA so that contraction
chunk c=(g*8+m) places k = 32*p + 8*g + m on partition p.  The x side
matches by PE-transposing stride-32 column slices of each 128-token x tile.
Per-chunk power-of-2 scalings (2^:4m on weights, 2^-4m via scaled identity
on x) keep bf16 exactness while letting nibble extraction be a single
tensor_scalar (mask-and + subtract) per chunk.
"""

import numpy as np

T = 4096
K = 4096
O = 11008
NCORES = 8
O_SHARD = O // NCORES  # 1376
P = 128
G = 4                  # u32 words per 16B DMA unit
QW = K // 8            # u32 words per packed row = 512
NK = K // P            # 32 contraction chunks
MM_N = 512             # matmul free-dim (one PSUM bank of f32)


def build_program(t_dim=T, k_dim=K, o_shard=O_SHARD):
    import concourse.bass as bass  # noqa: F401
    import concourse.mybir as mybir
    import concourse.bacc as bacc
    from concourse import tile
    from contextlib import ExitStack

    f32 = mybir.dt.float32
    bf16 = mybir.dt.bfloat16
    u32 = mybir.dt.uint32
    Alu = mybir.AluOpType

    qw = k_dim // 8
    nk = k_dim // P
    tt = t_dim // P
    assert qw // G == P, "weight staging must have 128 partitions"
    n_m = 8
    n_g = nk // n_m  # 4
    ocs = []
    o0 = 0
    while o0 < o_shard:
        ocs.append((o0, min(o_shard, o0 + MM_N)))
        o0 += MM_N

    nc = bacc.Bacc("TRN2", target_bir_lowering=False, debug=False)

    x_d = nc.dram_tensor("x", [t_dim, k_dim], f32, kind="ExternalInput")
    w_d = nc.dram_tensor("wp", [o_shard, qw], u32, kind="ExternalInput")
    sc_d = nc.dram_tensor("scb", [1, o_shard], f32, kind="ExternalInput")
    sz_d = nc.dram_tensor("szb", [1, o_shard], f32, kind="ExternalInput")
    y_d = nc.dram_tensor("y", [t_dim, o_shard], f32, kind="ExternalOutput")

    with tile.TileContext(nc) as tc, ExitStack() as ctx:
        const = ctx.enter_context(tc.tile_pool(name="const", bufs=1))
        wres = ctx.enter_context(tc.tile_pool(name="wres", bufs=1))
        wstage = ctx.enter_context(tc.tile_pool(name="wstage", bufs=1))
        xpool = ctx.enter_context(tc.tile_pool(name="xpool", bufs=2))
        xtpool = ctx.enter_context(tc.tile_pool(name="xtpool", bufs=2))
        spool = ctx.enter_context(tc.tile_pool(name="spool", bufs=2))
        opool = ctx.enter_context(tc.tile_pool(name="opool", bufs=4))
        tpsum = ctx.enter_context(tc.tile_pool(name="tpsum", bufs=2, space="PSUM"))
        mpsum = ctx.enter_context(tc.tile_pool(name="mpsum", bufs=2, space="PSUM"))

        ident = const.tile([P, P], f32, tag="ident")
        nc.gpsimd.memset(ident[:], 0.0)
        nc.gpsimd.affine_select(
            out=ident[:],
            in_=ident[:],
            compare_op=Alu.not_equal,
            fill=1.0,
            base=0,
            pattern=[[-1, P]],
            channel_multiplier=1,
        )

        # broadcast scale rows across partitions
        scb = const.tile([P, o_shard], f32, tag="scb")
        nc.sync.dma_start(out=scb[:], in_=sc_d.ap().to_broadcast((P, o_shard)))
        szb = const.tile([P, o_shard], f32, tag="szb")
        nc.sync.dma_start(out=szb[:], in_=sz_d.ap().to_broadcast((P, o_shard)))

        # ---- Phase W: load + unpack weights (resident in SBUF) ----
        # wsb[p, o, g] = u32 word (4p+g) of row o  (16B units in DRAM)
        wsb = wstage.tile([P, o_shard, G], u32)
        nc.sync.dma_start(out=wsb[:], in_=w_d.ap().rearrange("o (q g) -> q o g", g=G))
        wt = wres.tile([P, nk, o_shard], bf16)
        for c in range(nk):
            m, g = c % n_m, c // n_m
            # (word & (0xF<<4m)) == nib*2^4m  (bitwise can't cast: u32 tmp)
            wtmp = wstage.tile([P, o_shard], u32, tag="wtmp")
            nc.vector.tensor_single_scalar(
                out=wtmp[:],
                in_=wsb[:, :, g],
                scalar=0xF << (4 * m),
                op=Alu.bitwise_and,
            )
            # - 7.5*2^4m  ->  (nib - 7.5) * 2^4m, exact in bf16 (4 sig bits)
            nc.vector.tensor_single_scalar(
                out=wt[:, c, :],
                in_=wtmp[:],
                scalar=7.5 * (2.0 ** (4 * m)),
                op=Alu.subtract,
            )

        # ---- main loop over 128-token tiles ----
        for ti in range(tt):
            t0 = ti * P
            xsb = xpool.tile([P, k_dim], f32)
            nc.sync.dma_start(out=xsb[:], in_=x_d[t0 : t0 + P, :])
            # S[t] = sum_k x[t,k] (exact f32 x)
            s_col = spool.tile([P, 1], f32)
            nc.vector.reduce_sum(out=s_col[:], in_=xsb[:], axis=mybir.AxisListType.X)

            # transpose x into contraction-chunk layout (bf16, scaled 2^-4m)
            xt = xtpool.tile([P, nk, P], bf16)
            xv = xsb[:].rearrange("t (p r) -> t r p", r=32)
            for c in range(nk):
                m, g = c % n_m, c // n_m
                tp = tpsum.tile([P, P], f32)
                nc.tensor.transpose(tp[:], xv[:, 8 * g + m, :], ident[:])
                # copy PSUM -> SBUF bf16 with exact 2^-4m chunk scaling
                if c % 2 == 0:
                    nc.vector.tensor_scalar_mul(xt[:, c, :], tp[:], 2.0 ** (-4 * m))
                else:
                    nc.scalar.mul(xt[:, c, :], tp[:], 2.0 ** (-4 * m))

            for lo, hi in ocs:
                ow = hi - lo
                ps = mpsum.tile([P, MM_N], f32, tag=f"ps{lo}")
                for c in range(nk):
                    nc.tensor.matmul(
                        ps[:, :ow],
                        lhsT=xt[:, c, :],
                        rhs=wt[:, c, lo:hi],
                        start=(c == 0),
                        stop=(c == nk - 1),
                    )
                # y = ps*scale + S * (scale*(7.5-zero))
                t1 = opool.tile([P, MM_N], f32, tag="ep1")
                nc.vector.tensor_scalar_mul(t1[:, :ow], szb[:, lo:hi], s_col[:])
                yo = opool.tile([P, MM_N], f32, tag="ep2")
                nc.vector.tensor_mul(yo[:, :ow], ps[:, :ow], scb[:, lo:hi])
                nc.vector.tensor_add(yo[:, :ow], yo[:, :ow], t1[:, :ow])
                nc.sync.dma_start(out=y_d[t0 : t0 + P, lo:hi], in_=yo[:, :ow])

    nc.compile()
    return nc


_PROGRAM = None


def _get_program():
    global _PROGRAM
    if _PROGRAM is None:
        _PROGRAM = build_program()
    return _PROGRAM


def make_in_maps(x, weight_packed, scale, zero, o_shard=O_SHARD, ncores=NCORES):
    x = np.ascontiguousarray(np.asarray(x, dtype=np.float32))
    wp = np.ascontiguousarray(np.asarray(weight_packed, dtype=np.uint8))
    sc = np.asarray(scale, dtype=np.float32).reshape(-1)
    zr = np.asarray(zero, dtype=np.float32).reshape(-1)
    in_maps = []
    for c in range(ncores):
        o0 = c * o_shard
        wp32 = np.ascontiguousarray(wp[o0 : o0 + o_shard]).view(np.uint32)
        scs = np.ascontiguousarray(sc[o0 : o0 + o_shard].reshape(1, -1))
        szs = np.ascontiguousarray(
            (sc[o0 : o0 + o_shard] * (7.5 - zr[o0 : o0 + o_shard])).reshape(1, -1)
        )
        in_maps.append({"x": x, "wp": wp32, "scb": scs, "szb": szs})
    return in_maps


def kernel(x, weight_packed, scale, zero):
    from concourse.bass_utils import run_bass_kernel_spmd

    nc = _get_program()
    in_maps = make_in_maps(x, weight_packed, scale, zero)
    res = run_bass_kernel_spmd(nc, in_maps, core_ids=list(range(NCORES)))
    return np.concatenate([r["y"] for r in res.results], axis=1)


# revision 8
# speedup vs baseline: 1.5814x; 1.5814x over previous
"""Int4 dequant matmul kernel for Trainium2 (8 NeuronCores, tensor-parallel).

Computes y = x @ W.T where W = (nibbles(weight_packed) - zero) * scale,
x: (4096, 4096) f32, weight_packed: (11008, 2048) u8, y: (4096, 11008) f32.

Sharding: column-parallel over out_features (1376 per core), x replicated.

Math:  y[t,o] = scale[o] * (sum_k x[t,k]*(n[o,k]-7.5)) + scale[o]*(7.5-zero[o])*S[t]
with S[t] = sum_k x[t,k].  The matmul runs in bf16 with EXACT weights
(n-7.5 is representable in bf16), so the only error source is bf16
rounding of x against zero-mean weights (~1.3e-3 L2 rel).

Layout: the host repacks the int4 weights (a pure bit permutation) into
u32 words wsb[p, g, o] whose nibble m holds n[o, 128*(8g+m) + p], so one
contiguous DMA stages them and contraction chunk c=8g+m lives on
partition p = k - 128c.  The x side matches via a single SBUF->SBUF
xbar DMA-transpose per 128-token tile: [128t, 4096k]bf16 ->
[p, c, t] with k = 128c + p.  On-chip nibble extraction is two DVE ops
per chunk: (word & (0xF<<4m)) then (*2^-4m - 7.5) with cast to bf16.
"""

import numpy as np

T = 4096
K = 4096
O = 11008
NCORES = 8
O_SHARD = O // NCORES  # 1376
P = 128
G = 4                  # u32 words per packed group (8 nibbles each)
NK = K // P            # 32 contraction chunks
MM_N = 512             # matmul free-dim (one PSUM bank of f32)


def build_program(t_dim=T, k_dim=K, o_shard=O_SHARD):
    import concourse.mybir as mybir
    import concourse.bacc as bacc
    from concourse import tile
    from contextlib import ExitStack

    f32 = mybir.dt.float32
    bf16 = mybir.dt.bfloat16
    u32 = mybir.dt.uint32
    Alu = mybir.AluOpType

    nk = k_dim // P
    tt = t_dim // P
    n_m = 8
    n_g = nk // n_m  # 4
    ocs = []
    o0 = 0
    while o0 < o_shard:
        ocs.append((o0, min(o_shard, o0 + MM_N)))
        o0 += MM_N

    nc = bacc.Bacc("TRN2", target_bir_lowering=False, debug=False)

    x_d = nc.dram_tensor("x", [t_dim, k_dim], f32, kind="ExternalInput")
    w_d = nc.dram_tensor("wp", [P, n_g, o_shard], u32, kind="ExternalInput")
    sc_d = nc.dram_tensor("scb", [1, o_shard], f32, kind="ExternalInput")
    sz_d = nc.dram_tensor("szb", [1, o_shard], f32, kind="ExternalInput")
    y_d = nc.dram_tensor("y", [t_dim, o_shard], f32, kind="ExternalOutput")

    with tile.TileContext(nc) as tc, ExitStack() as ctx:
        const = ctx.enter_context(tc.tile_pool(name="const", bufs=1))
        wres = ctx.enter_context(tc.tile_pool(name="wres", bufs=1))
        wstage = ctx.enter_context(tc.tile_pool(name="wstage", bufs=1))
        xpool = ctx.enter_context(tc.tile_pool(name="xpool", bufs=2))
        xbpool = ctx.enter_context(tc.tile_pool(name="xbpool", bufs=2))
        xtpool = ctx.enter_context(tc.tile_pool(name="xtpool", bufs=2))
        spool = ctx.enter_context(tc.tile_pool(name="spool", bufs=2))
        opool = ctx.enter_context(tc.tile_pool(name="opool", bufs=4))
        mpsum = ctx.enter_context(tc.tile_pool(name="mpsum", bufs=2, space="PSUM"))

        # broadcast scale rows across partitions
        scb = const.tile([P, o_shard], f32, tag="scb")
        nc.sync.dma_start(out=scb[:], in_=sc_d.ap().to_broadcast((P, o_shard)))
        szb = const.tile([P, o_shard], f32, tag="szb")
        nc.sync.dma_start(out=szb[:], in_=sz_d.ap().to_broadcast((P, o_shard)))

        # ---- Phase W: load + unpack weights (resident in SBUF) ----
        wsb = wstage.tile([P, n_g, o_shard], u32)
        nc.sync.dma_start(out=wsb[:], in_=w_d.ap())
        wt = wres.tile([P, nk, o_shard], bf16)
        for c in range(nk):
            g, m = c // n_m, c % n_m
            wtmp = wstage.tile([P, o_shard], u32, tag="wtmp")
            nc.vector.tensor_single_scalar(
                out=wtmp[:],
                in_=wsb[:, g, :],
                scalar=0xF << (4 * m),
                op=Alu.bitwise_and,
            )
            # (nib<<4m) * 2^-4m - 7.5  (arith ops may cast u32->bf16)
            nc.vector.tensor_scalar(
                out=wt[:, c, :],
                in0=wtmp[:],
                scalar1=2.0 ** (-4 * m),
                scalar2=7.5,
                op0=Alu.mult,
                op1=Alu.subtract,
            )

        # ---- main loop over 128-token tiles ----
        for ti in range(tt):
            t0 = ti * P
            xsb = xpool.tile([P, k_dim], f32)
            nc.sync.dma_start(out=xsb[:], in_=x_d[t0 : t0 + P, :])
            # S[t] = sum_k x[t,k] (exact f32 x) on DVE
            s_col = spool.tile([P, 1], f32)
            nc.vector.reduce_sum(out=s_col[:], in_=xsb[:], axis=mybir.AxisListType.X)
            # cast to bf16 on ScalarE (ACT otherwise idle)
            xb = xbpool.tile([P, k_dim], bf16)
            nc.scalar.copy(out=xb[:], in_=xsb[:])
            # one xbar transpose: xt[p, c, t] = xb[t, 128c + p]
            xt = xtpool.tile([P, nk, P], bf16)
            nc.sync.dma_start(out=xt[:], in_=xb[:], transpose=True)

            for lo, hi in ocs:
                ow = hi - lo
                ps = mpsum.tile([P, MM_N], f32, tag=f"ps{lo}")
                for c in range(nk):
                    nc.tensor.matmul(
                        ps[:, :ow],
                        lhsT=xt[:, c, :],
                        rhs=wt[:, c, lo:hi],
                        start=(c == 0),
                        stop=(c == nk - 1),
                    )
                # y = ps*scale + S * (scale*(7.5-zero))
                t1 = opool.tile([P, MM_N], f32, tag="ep1")
                nc.vector.tensor_scalar_mul(t1[:, :ow], szb[:, lo:hi], s_col[:])
                yo = opool.tile([P, MM_N], f32, tag="ep2")
                nc.vector.tensor_mul(yo[:, :ow], ps[:, :ow], scb[:, lo:hi])
                nc.vector.tensor_add(yo[:, :ow], yo[:, :ow], t1[:, :ow])
                nc.sync.dma_start(out=y_d[t0 : t0 + P, lo:hi], in_=yo[:, :ow])

    nc.compile()
    return nc


_PROGRAM = None


def _get_program():
    global _PROGRAM
    if _PROGRAM is None:
        _PROGRAM = build_program()
    return _PROGRAM


def repack_weights(wp, o_shard):
    """(O, K/2) u8 -> per-full-array [O, P, G] u32 with nibble m of word
    [o, p, g] = nib[o, 128*(8g+m) + p].  Pure bit permutation."""
    O_full, kb = wp.shape
    k_dim = kb * 2
    nib = np.empty((O_full, k_dim), dtype=np.uint8)
    nib[:, 0::2] = wp & 0x0F
    nib[:, 1::2] = wp >> 4
    nk = k_dim // P
    n_g = nk // 8
    # k = 128c + p, c = 8g + m  ->  [o, g, m, p]
    v = nib.reshape(O_full, n_g, 8, P)
    word = np.zeros((O_full, n_g, P), dtype=np.uint32)
    for m in range(8):
        word |= v[:, :, m, :].astype(np.uint32) << (4 * m)
    return word  # [o, g, p]


def make_in_maps(x, weight_packed, scale, zero, o_shard=O_SHARD, ncores=NCORES):
    x = np.ascontiguousarray(np.asarray(x, dtype=np.float32))
    wp = np.ascontiguousarray(np.asarray(weight_packed, dtype=np.uint8))
    sc = np.asarray(scale, dtype=np.float32).reshape(-1)
    zr = np.asarray(zero, dtype=np.float32).reshape(-1)
    word = repack_weights(wp, o_shard)  # [o, g, p]
    in_maps = []
    for c in range(ncores):
        o0 = c * o_shard
        wps = np.ascontiguousarray(
            word[o0 : o0 + o_shard].transpose(2, 1, 0)
        )  # [p, g, o]
        scs = np.ascontiguousarray(sc[o0 : o0 + o_shard].reshape(1, -1))
        szs = np.ascontiguousarray(
            (sc[o0 : o0 + o_shard] * (7.5 - zr[o0 : o0 + o_shard])).reshape(1, -1)
        )
        in_maps.append({"x": x, "wp": wps, "scb": scs, "szb": szs})
    return in_maps


def kernel(x, weight_packed, scale, zero):
    from concourse.bass_utils import run_bass_kernel_spmd

    nc = _get_program()
    in_maps = make_in_maps(x, weight_packed, scale, zero)
    res = run_bass_kernel_spmd(nc, in_maps, core_ids=list(range(NCORES)))
    return np.concatenate([r["y"] for r in res.results], axis=1)
